# revision 33
# baseline (speedup 1.0000x reference)
"""Trainium2 Bass kernel for a 16-step neural cellular automaton (BasicNCA).

Reference semantics (per step):
    c   = conv3x3(x, k, SAME)                    # 1 channel
    g   = exp(-(c-1)^2)
    h   = relu(g*w1 + b1); o = sigmoid(h@w2)     # pointwise 1->10->1 MLP
    x  += o - 0.5
Output: all 17 states stacked, [17, 16, 1, 512, 512] f32.

Design (evolution of the previous 291us kernel; see trace analysis):
 * The pointwise chain delta(c) = sigmoid(P(exp(-(c-1)^2))) - 0.5 is an exact
   function of u = exp(-(c-1)^2).  Fitting a quadratic in the Gaussian
        delta(c) ~= c2*(u - r1)*(u - r2),  u = (2/sqrt(pi))*exp(-(s(c-1))^2)
   (refit on the host from the actual w1/b1/w2) has max err 2.2e-3 -- 2.5x
   better than the old Abs+Gelu two-pass form -- and needs only ONE ScalarE
   pass (ActivationFunctionType.Derivative_Erf == (2/sqrt(pi))e^{-x^2}) plus
   two fp16 VectorE ops (tensor_scalar, scalar_tensor_tensor).  The factored
   form makes the VectorE output the FULL delta, so the incremental conv
   needs no constant-drift bookkeeping at all.
 * The conv state c lives permanently in PSUM (all 8 banks) and is updated
   incrementally by the TensorEngine: c += conv3x3(delta) in fp16, as
   3 banded (tridiagonal) 128x128 matmuls per row-tile plus one 6-row halo
   matmul.  delta rows are stored with a 514-element tile pitch whose 2 zero
   pad columns implement SAME-padding column edges for the +-1 shifted
   matmuls, so all matmuls are full 512-column and halo DMAs write full
   unclipped rows.
 * The x update x += delta runs on the otherwise idle Pool/GpSimd engine,
   halo DMAs are split across the sync and pool rings, and the output write
   rides the scalar ring.
 * Sharding: pure data parallel, 2 images per NeuronCore across 8 cores.
"""

import math

import numpy as np

P = 128          # partitions
W = 512          # image width (= free size per row-tile)
TPI = 4          # row-tiles per image (4 * 128 = 512 rows)
NIMG = 2         # images per core
NT = TPI * NIMG  # row-tiles per core
NCORES = 8
FREE = NT * W    # free size of full-state SBUF tensors (x, u)
PITCH = W + 2    # padded tile pitch for delta / halo tensors
FREEP = NT * PITCH + 2  # +2: slack so shifted tile views stay in bounds

# Fitted on the reference setup_inputs() weights; full-trajectory rel err
# 1.5e-3 in a bit-faithful numpy simulation of this kernel.
#   delta(c) ~= c0 + u*(c1 + c2*u), u = (2/sqrt(pi))*exp(-(s*(c-1))^2)
_DEFAULT_PARAMS = (1.08490766, 0.02218426, 0.16743472, -0.01551842)

_NC_CACHE = {}
LAST_RESULTS = None

_K2 = 2.0 / math.sqrt(math.pi)


# --------------------------------------------------------------------------
# Host-side scalar-map fitting
# --------------------------------------------------------------------------

def _delta_exact(c, w1, b1, w2):
    g = np.exp(-(c - 1.0) ** 2)
    z = g[..., None] * w1.reshape(-1) + b1.reshape(-1)
    pv = (np.maximum(z, 0.0) * w2.reshape(-1)).sum(-1)
    return 1.0 / (1.0 + np.exp(-pv)) - 0.5


def _model(p, c):
    s, c0, c1, c2 = p
    u = _K2 * np.exp(-(s * (c - 1.0)) ** 2)
    return c0 + u * (c1 + c2 * u)


def _get_params(w1, b1, w2):
    grid = np.linspace(-26.0, 26.0, 40001)
    target = _delta_exact(grid, w1, b1, w2)
    p0 = np.array(_DEFAULT_PARAMS)
    err0 = float(np.abs(_model(p0, grid) - target).max())
    if err0 < 4e-3:
        return tuple(p0)
    # Weights differ from the ones this kernel was tuned on -- refit.
    tail = float(target[0])
    best = (err0, p0)
    try:
        from scipy.optimize import least_squares
        for s0 in (0.6, 1.0, 1.6):
            peak = float(target[grid.searchsorted(1.0)])
            c1g = (peak - tail) / _K2
            init = [s0, tail, c1g, 0.0]
            try:
                sol = least_squares(lambda p: _model(p, grid) - target,
                                    init, max_nfev=8000)
                e = float(np.abs(_model(sol.x, grid) - target).max())
                if e < best[0]:
                    best = (e, sol.x)
            except Exception:
                pass
    except Exception:
        pass
    return tuple(float(v) for v in best[1])


# --------------------------------------------------------------------------
# Bass program
# --------------------------------------------------------------------------

def _build_nc(kk, params, steps):
    from concourse import bacc, mybir, tile

    f32 = mybir.dt.float32
    f16 = mybir.dt.float16
    AF = mybir.ActivationFunctionType
    OP = mybir.AluOpType

    s_, c0_, c1_, c2_ = [float(v) for v in params]
    # delta = c0 + c1*u + c2*u^2 = c2*(u - r1)*(u - r2); complex roots can
    # only arise from a degenerate refit -- nudge c0 until real.
    disc = c1_ * c1_ - 4.0 * c2_ * c0_
    if disc < 0.0:
        c0_ = c1_ * c1_ / (4.0 * c2_) * 0.999
        disc = c1_ * c1_ - 4.0 * c2_ * c0_
    r1_ = (-c1_ + math.sqrt(disc)) / (2.0 * c2_)
    r2_ = (-c1_ - math.sqrt(disc)) / (2.0 * c2_)
    # complete-the-square form for the Square-ACT path:
    #   delta = c2*(u - m)^2 - c2*d^2
    m_ = (r1_ + r2_) / 2.0
    cd2_ = -c2_ * ((r1_ - r2_) / 2.0) ** 2

    kk = np.asarray(kk, np.float32).reshape(3, 3)
    kk16 = kk.astype(np.float16)

    nc = bacc.Bacc("TRN2", target_bir_lowering=False, debug=False,
                   num_devices=NCORES)
    x_in = nc.dram_tensor("x", [NIMG, W, W], f32, kind="ExternalInput")
    out = nc.dram_tensor("out", [steps + 1, NIMG, W, W], f32,
                         kind="ExternalOutput")

    # ---- host-built constants --------------------------------------------
    def banded(kcol):
        # lhsT[qrow, prow]: input row q feeds output row p with kernel row
        # index 1 + (q - p).  out[p,c] = sum_q lhsT[q,p] * rhs[q,c].
        m = np.zeros((P, P), kcol.dtype)
        for dr in (-1, 0, 1):
            for p in range(P):
                q = p + dr
                if 0 <= q < P:
                    m[q, p] = kcol[1 + dr]
        return m

    a16_h = [nc.inline_tensor(banded(kk16[:, j]), name=f"A16{j}")
             for j in range(3)]
    z16_h = nc.inline_tensor(np.zeros((P, P), np.float16), name="Z16")

    # Shared 6-row halo lhsT: rows 0-2 above-halo (k[0,j] -> out row 0),
    # rows 3-5 below-halo (k[2,j] -> out row 127).
    hm = np.zeros((6, P), np.float16)
    for j in range(3):
        hm[j, 0] = kk16[0, j]
        hm[3 + j, P - 1] = kk16[2, j]
    h16_h = nc.inline_tensor(hm, name="H16")

    # ---- on-chip tensors -------------------------------------------------
    # 4-deep x rotation: the emit DMA of state s has 3 full steps to drain
    # before its buffer is rewritten, so the x update never blocks on it.
    xb = [nc.alloc_sbuf_tensor(f"xs{i}", [P, FREE], f32) for i in range(4)]
    ub = nc.alloc_sbuf_tensor("u16", [P, FREE], f16)
    # dl / h16 double-buffered by step parity so this step's delta writes
    # never wait on the previous conv burst's reads
    dlb = [nc.alloc_sbuf_tensor(f"delta{i}", [P, FREEP], f16)
           for i in range(2)]
    h16b = [nc.alloc_sbuf_tensor(f"halo16_{i}", [6, FREEP], f16)
            for i in range(2)]
    wa16 = [nc.alloc_sbuf_tensor(f"wa16{j}", [P, P], f16) for j in range(3)]
    wz16 = nc.alloc_sbuf_tensor("wz16", [P, P], f16)
    wh16 = nc.alloc_sbuf_tensor("wh16", [6, P], f16)

    CW = 2 * W  # pointwise chunk = one PSUM pair (2 tiles)

    def wbase(bt):
        return bt * PITCH + 1

    with tile.TileContext(nc) as tc:
        with (
            tc.tile_pool(name="psum", bufs=1, space="PSUM") as pp,
            tc.tile_pool(name="tmp", bufs=3) as pool,
        ):
            # four PSUM tensors of 2 banks each (tile pairs): fine-grained
            # dependency domains -> short per-pair pipeline loops
            cps = [pp.tile([P, CW], f32, tag=f"c{g}", name=f"c{g}")
                   for g in range(4)]

            # ---------------- init ----------------
            bias_act = nc.alloc_sbuf_tensor("bias_act", [P, 1], f32)
            nc.vector.memset(bias_act.ap(), -s_)
            bias_m = nc.alloc_sbuf_tensor("bias_m", [P, 1], f32)
            nc.vector.memset(bias_m.ap(), -m_)
            for j in range(3):
                nc.sync.dma_start(out=wa16[j].ap(), in_=a16_h[j].ap())
            nc.sync.dma_start(out=wh16.ap(), in_=h16_h.ap())
            nc.sync.dma_start(out=wz16.ap(), in_=z16_h.ap())
            for i in range(2):
                nc.vector.memset(h16b[i].ap(), 0.0)
                nc.vector.memset(dlb[i].ap(), 0.0)

            # load x0, emit state 0
            xv_dram = x_in.rearrange("b (t p) c -> p b t c", p=P)
            nc.sync.dma_start(
                out=xb[0].ap().rearrange("p (b t c) -> p b t c", b=NIMG, t=TPI),
                in_=xv_dram)
            out_v = out.rearrange("s b (t p) c -> p s b t c", p=P)

            def emit_state(x_t, s):
                nc.sync.dma_start(
                    out=out_v[:, s:s + 1],
                    in_=x_t.ap().rearrange(
                        "p (b t c) -> p b t c", b=NIMG, t=TPI).unsqueeze(1))

            emit_state(xb[0], 0)

            CS = 3 * PITCH - 2  # contiguous span of 3 tile windows + pads

            def dl_tile(par, t):
                # [P, W] contiguous view of tile t's delta window
                start = wbase(t)
                return dlb[par].ap()[:, start:start + W]

            def halo_above(eng, par, b):
                # above-halo of tiles 1..3 <- row 127 of tiles 0..2, as one
                # fully contiguous 1-D copy per shift: the src pads are
                # permanent zeros and land in dst pads / shifted edges,
                # implementing SAME padding exactly.
                s0 = wbase(b * TPI)
                for j in range(3):
                    dc = j - 1
                    eng.dma_start(
                        out=h16b[par].ap()[j:j + 1,
                                           s0 + PITCH - dc:s0 + PITCH - dc + CS],
                        in_=dlb[par].ap()[P - 1:P, s0:s0 + CS])

            def halo_below(eng, par, b):
                # below-halo of tiles 0..2 <- row 0 of tiles 1..3
                s0 = wbase(b * TPI)
                for j in range(3):
                    dc = j - 1
                    eng.dma_start(
                        out=h16b[par].ap()[3 + j:4 + j, s0 - dc:s0 - dc + CS],
                        in_=dlb[par].ap()[0:1, s0 + PITCH:s0 + PITCH + CS])

            def banded_mms(par, pr, start):
                # c[pair pr] += row-banded conv terms of its 2 tiles
                cp = cps[pr]
                for j in (1, 0, 2):
                    dc = j - 1
                    for t in (2 * pr, 2 * pr + 1):
                        ts0, cs0 = wbase(t) + dc, (t % 2) * W
                        nc.tensor.matmul(out=cp[:, cs0:cs0 + W],
                                         lhsT=wa16[j].ap(),
                                         rhs=dlb[par].ap()[:, ts0:ts0 + W],
                                         start=start and j == 1, stop=False)

            def halo_mms(par, pr):
                # boundary-row contributions for pair pr's tiles
                cp = cps[pr]
                for t in (2 * pr, 2 * pr + 1):
                    ts0, cs0 = wbase(t), (t % 2) * W
                    nc.tensor.matmul(out=cp[:, cs0:cs0 + W],
                                     lhsT=wh16.ap(),
                                     rhs=h16b[par].ap()[:, ts0:ts0 + W],
                                     start=False, stop=True)

            def warm_mms(n):
                # zero-weight matmuls into cps[3]: keep the PE clock ramped
                # across the inter-burst gap without touching real state
                for _ in range(n):
                    nc.tensor.matmul(out=cps[3][:, 0:P],
                                     lhsT=wz16.ap(), rhs=wa16[0].ap(),
                                     start=False, stop=False,
                                     skip_group_check=True)

            # fp16 conv of the initial state into PSUM via the delta buffer
            # (parity 1 = "step -1", so step 0's delta writes don't collide)
            for t in range(NT):
                nc.vector.tensor_copy(
                    out=dl_tile(1, t),
                    in_=xb[0].ap()[:, t * W:(t + 1) * W])
            for b in range(NIMG):
                halo_above(nc.sync, 1, b)
                halo_below(nc.sync, 1, b)
            for pr in range(4):
                banded_mms(1, pr, True)
            for pr in range(4):
                halo_mms(1, pr)

            # ---------------- steps ----------------
            def x_update(par, x_cur, x_new, t, eng):
                # x_new = x + delta (f32; 1X on DVE, slower on Pool)
                eng.tensor_tensor(
                    out=x_new.ap()[:, t * W:(t + 1) * W],
                    in0=x_cur.ap()[:, t * W:(t + 1) * W],
                    in1=dl_tile(par, t),
                    op=OP.add)

            for s in range(steps):
                par = s % 2
                x_cur, x_new = xb[s % 4], xb[(s + 1) % 4]
                last = s == steps - 1
                warm_mms(24)

                for pr in range(4):
                    fs = pr * CW
                    u_s = ub.ap()[:, fs:fs + CW]
                    nc.scalar.activation(
                        out=u_s, in_=cps[pr][:, :],
                        func=AF.Derivative_Erf, bias=bias_act.ap(), scale=s_)
                    if pr % 2 == 0:
                        # factored path, all ops in DVE 4x/2x modes:
                        #   dl = [(u-r1)*c2] * [u-r2]
                        w1 = pool.tile([P, CW], f16, tag="w1",
                                       name=f"w1_{s}_{pr}")
                        w2 = pool.tile([P, CW], f16, tag="w2",
                                       name=f"w2_{s}_{pr}")
                        nc.vector.tensor_scalar(
                            out=w1[:], in0=u_s, scalar1=r1_, scalar2=c2_,
                            op0=OP.subtract, op1=OP.mult)
                        nc.vector.tensor_scalar(
                            out=w2[:], in0=u_s, scalar1=r2_, scalar2=1.0,
                            op0=OP.subtract, op1=OP.mult)
                        for ti in range(2):
                            t = 2 * pr + ti
                            nc.vector.tensor_tensor(
                                out=dl_tile(par, t),
                                in0=w1[:, ti * W:(ti + 1) * W],
                                in1=w2[:, ti * W:(ti + 1) * W],
                                op=OP.mult)
                    else:
                        # Square-ACT path (balances ScalarE vs VectorE):
                        #   y = (u-m)^2 on ScalarE, dl = c2*y - c2*d^2
                        y = pool.tile([P, CW], f16, tag="y",
                                      name=f"y_{s}_{pr}")
                        nc.scalar.activation(
                            out=y[:], in_=u_s,
                            func=AF.Square, bias=bias_m.ap(), scale=1.0)
                        for ti in range(2):
                            t = 2 * pr + ti
                            nc.vector.tensor_scalar(
                                out=dl_tile(par, t),
                                in0=y[:, ti * W:(ti + 1) * W],
                                scalar1=c2_, scalar2=cd2_,
                                op0=OP.mult, op1=OP.add)
                    if pr == 1:
                        for t in range(0, 4):
                            x_update(par, x_cur, x_new, t,
                                     nc.vector if t < 2 else nc.gpsimd)
                        if not last:
                            halo_above(nc.sync, par, 0)
                            halo_below(nc.sync, par, 0)
                    elif pr == 3:
                        for t in range(4, 8):
                            x_update(par, x_cur, x_new, t,
                                     nc.vector if t < 6 else nc.gpsimd)
                        if not last:
                            halo_above(nc.sync, par, 1)
                            halo_below(nc.scalar, par, 1)
                emit_state(x_new, s + 1)
                if not last:
                    # image-0 pairs close first so the next step's chain
                    # (ACT p0 -> DVE -> banded p0) overlaps this burst's tail
                    for pr in (0, 1):
                        banded_mms(par, pr, False)
                    for pr in (0, 1):
                        halo_mms(par, pr)
                    for pr in (2, 3):
                        banded_mms(par, pr, False)
                    for pr in (2, 3):
                        halo_mms(par, pr)

    nc.compile()
    return nc


# --------------------------------------------------------------------------
# Entry point
# --------------------------------------------------------------------------

def kernel(x, k, w1, b1, w2, steps):
    global LAST_RESULTS
    steps = int(np.asarray(steps))
    x = np.asarray(x, np.float32)
    k = np.asarray(k, np.float32).reshape(3, 3)
    B = x.shape[0]
    assert B == NIMG * NCORES and x.shape[-2:] == (W, W)

    params = _get_params(np.asarray(w1, np.float64), np.asarray(b1, np.float64),
                         np.asarray(w2, np.float64))

    key = (steps, k.tobytes(), tuple(params))
    nc = _NC_CACHE.get(key)
    if nc is None:
        nc = _build_nc(k, params, steps)
        _NC_CACHE.clear()
        _NC_CACHE[key] = nc

    xs = np.ascontiguousarray(x.reshape(B, W, W))
    in_maps = [{"x": np.ascontiguousarray(xs[NIMG * i:NIMG * (i + 1)])}
               for i in range(NCORES)]

    from concourse.bass_utils import run_bass_kernel_spmd
    res = run_bass_kernel_spmd(nc, in_maps, core_ids=list(range(NCORES)))
    LAST_RESULTS = res

    full = np.concatenate([np.asarray(r["out"]) for r in res.results], axis=1)
    return np.ascontiguousarray(full[:, :, None].astype(np.float32))


if __name__ == "__main__":
    rng = np.random.default_rng(0)
    x = rng.standard_normal((16, 1, W, W), dtype=np.float32)
    k = rng.standard_normal((1, 1, 3, 3)).astype(np.float32)
    w1 = (rng.standard_normal((10, 1)) * 0.5).astype(np.float32)
    b1 = (rng.standard_normal((10,)) * 0.1).astype(np.float32)
    w2 = (rng.standard_normal((1, 10)) * 0.5).astype(np.float32)
    out = kernel(x=x, k=k, w1=w1, b1=b1, w2=w2, steps=16)
    print("out", out.shape, out.dtype)


# revision 35
# speedup vs baseline: 1.0391x; 1.0391x over previous
"""Trainium2 Bass kernel for a 16-step neural cellular automaton (BasicNCA).

Reference semantics (per step):
    c   = conv3x3(x, k, SAME)                    # 1 channel
    g   = exp(-(c-1)^2)
    h   = relu(g*w1 + b1); o = sigmoid(h@w2)     # pointwise 1->10->1 MLP
    x  += o - 0.5
Output: all 17 states stacked, [17, 16, 1, 512, 512] f32.

Design (evolution of the previous 291us kernel; see trace analysis):
 * The pointwise chain delta(c) = sigmoid(P(exp(-(c-1)^2))) - 0.5 is an exact
   function of u = exp(-(c-1)^2).  Fitting a quadratic in the Gaussian
        delta(c) ~= c2*(u - r1)*(u - r2),  u = (2/sqrt(pi))*exp(-(s(c-1))^2)
   (refit on the host from the actual w1/b1/w2) has max err 2.2e-3 -- 2.5x
   better than the old Abs+Gelu two-pass form -- and needs only ONE ScalarE
   pass (ActivationFunctionType.Derivative_Erf == (2/sqrt(pi))e^{-x^2}) plus
   two fp16 VectorE ops (tensor_scalar, scalar_tensor_tensor).  The factored
   form makes the VectorE output the FULL delta, so the incremental conv
   needs no constant-drift bookkeeping at all.
 * The conv state c lives permanently in PSUM (all 8 banks) and is updated
   incrementally by the TensorEngine: c += conv3x3(delta) in fp16, as
   3 banded (tridiagonal) 128x128 matmuls per row-tile plus one 6-row halo
   matmul.  delta rows are stored with a 514-element tile pitch whose 2 zero
   pad columns implement SAME-padding column edges for the +-1 shifted
   matmuls, so all matmuls are full 512-column and halo DMAs write full
   unclipped rows.
 * The x update x += delta runs on the otherwise idle Pool/GpSimd engine,
   halo DMAs are split across the sync and pool rings, and the output write
   rides the scalar ring.
 * Sharding: pure data parallel, 2 images per NeuronCore across 8 cores.
"""

import math

import numpy as np

P = 128          # partitions
W = 512          # image width (= free size per row-tile)
TPI = 4          # row-tiles per image (4 * 128 = 512 rows)
NIMG = 2         # images per core
NT = TPI * NIMG  # row-tiles per core
NCORES = 8
FREE = NT * W    # free size of full-state SBUF tensors (x, u)
PITCH = W + 2    # padded tile pitch for delta / halo tensors
FREEP = NT * PITCH + 2  # +2: slack so shifted tile views stay in bounds

# Fitted on the reference setup_inputs() weights; full-trajectory rel err
# 1.5e-3 in a bit-faithful numpy simulation of this kernel.
#   delta(c) ~= c0 + u*(c1 + c2*u), u = (2/sqrt(pi))*exp(-(s*(c-1))^2)
_DEFAULT_PARAMS = (1.08490766, 0.02218426, 0.16743472, -0.01551842)

_NC_CACHE = {}
LAST_RESULTS = None

_K2 = 2.0 / math.sqrt(math.pi)


# --------------------------------------------------------------------------
# Host-side scalar-map fitting
# --------------------------------------------------------------------------

def _delta_exact(c, w1, b1, w2):
    g = np.exp(-(c - 1.0) ** 2)
    z = g[..., None] * w1.reshape(-1) + b1.reshape(-1)
    pv = (np.maximum(z, 0.0) * w2.reshape(-1)).sum(-1)
    return 1.0 / (1.0 + np.exp(-pv)) - 0.5


def _model(p, c):
    s, c0, c1, c2 = p
    u = _K2 * np.exp(-(s * (c - 1.0)) ** 2)
    return c0 + u * (c1 + c2 * u)


def _get_params(w1, b1, w2):
    grid = np.linspace(-26.0, 26.0, 40001)
    target = _delta_exact(grid, w1, b1, w2)
    p0 = np.array(_DEFAULT_PARAMS)
    err0 = float(np.abs(_model(p0, grid) - target).max())
    if err0 < 4e-3:
        return tuple(p0)
    # Weights differ from the ones this kernel was tuned on -- refit.
    tail = float(target[0])
    best = (err0, p0)
    try:
        from scipy.optimize import least_squares
        for s0 in (0.6, 1.0, 1.6):
            peak = float(target[grid.searchsorted(1.0)])
            c1g = (peak - tail) / _K2
            init = [s0, tail, c1g, 0.0]
            try:
                sol = least_squares(lambda p: _model(p, grid) - target,
                                    init, max_nfev=8000)
                e = float(np.abs(_model(sol.x, grid) - target).max())
                if e < best[0]:
                    best = (e, sol.x)
            except Exception:
                pass
    except Exception:
        pass
    return tuple(float(v) for v in best[1])


# --------------------------------------------------------------------------
# Bass program
# --------------------------------------------------------------------------

def _build_nc(kk, params, steps):
    from concourse import bacc, mybir, tile

    f32 = mybir.dt.float32
    f16 = mybir.dt.float16
    AF = mybir.ActivationFunctionType
    OP = mybir.AluOpType

    s_, c0_, c1_, c2_ = [float(v) for v in params]
    # delta = c0 + c1*u + c2*u^2 = c2*(u - r1)*(u - r2); complex roots can
    # only arise from a degenerate refit -- nudge c0 until real.
    disc = c1_ * c1_ - 4.0 * c2_ * c0_
    if disc < 0.0:
        c0_ = c1_ * c1_ / (4.0 * c2_) * 0.999
        disc = c1_ * c1_ - 4.0 * c2_ * c0_
    r1_ = (-c1_ + math.sqrt(disc)) / (2.0 * c2_)
    r2_ = (-c1_ - math.sqrt(disc)) / (2.0 * c2_)
    # complete-the-square form for the Square-ACT path:
    #   delta = c2*(u - m)^2 - c2*d^2
    m_ = (r1_ + r2_) / 2.0
    cd2_ = -c2_ * ((r1_ - r2_) / 2.0) ** 2

    kk = np.asarray(kk, np.float32).reshape(3, 3)
    kk16 = kk.astype(np.float16)

    nc = bacc.Bacc("TRN2", target_bir_lowering=False, debug=False,
                   num_devices=NCORES)
    x_in = nc.dram_tensor("x", [NIMG, W, W], f32, kind="ExternalInput")
    out = nc.dram_tensor("out", [steps + 1, NIMG, W, W], f32,
                         kind="ExternalOutput")

    # ---- host-built constants --------------------------------------------
    def banded(kcol):
        # lhsT[qrow, prow]: input row q feeds output row p with kernel row
        # index 1 + (q - p).  out[p,c] = sum_q lhsT[q,p] * rhs[q,c].
        m = np.zeros((P, P), kcol.dtype)
        for dr in (-1, 0, 1):
            for p in range(P):
                q = p + dr
                if 0 <= q < P:
                    m[q, p] = kcol[1 + dr]
        return m

    a16_h = [nc.inline_tensor(banded(kk16[:, j]), name=f"A16{j}")
             for j in range(3)]
    z16_h = nc.inline_tensor(np.zeros((P, P), np.float16), name="Z16")

    # Shared 6-row halo lhsT: rows 0-2 above-halo (k[0,j] -> out row 0),
    # rows 3-5 below-halo (k[2,j] -> out row 127).
    hm = np.zeros((6, P), np.float16)
    for j in range(3):
        hm[j, 0] = kk16[0, j]
        hm[3 + j, P - 1] = kk16[2, j]
    h16_h = nc.inline_tensor(hm, name="H16")

    # ---- on-chip tensors -------------------------------------------------
    # 4-deep x rotation: the emit DMA of state s has 3 full steps to drain
    # before its buffer is rewritten, so the x update never blocks on it.
    xb = [nc.alloc_sbuf_tensor(f"xs{i}", [P, FREE], f32) for i in range(4)]
    ub = nc.alloc_sbuf_tensor("u16", [P, FREE], f16)
    # dl / h16 double-buffered by step parity so this step's delta writes
    # never wait on the previous conv burst's reads
    dlb = [nc.alloc_sbuf_tensor(f"delta{i}", [P, FREEP], f16)
           for i in range(2)]
    h16b = [nc.alloc_sbuf_tensor(f"halo16_{i}", [6, FREEP], f16)
            for i in range(2)]
    wa16 = [nc.alloc_sbuf_tensor(f"wa16{j}", [P, P], f16) for j in range(3)]
    wz16 = nc.alloc_sbuf_tensor("wz16", [P, P], f16)
    wh16 = nc.alloc_sbuf_tensor("wh16", [6, P], f16)

    CW = 2 * W  # pointwise chunk = one PSUM pair (2 tiles)

    def wbase(bt):
        return bt * PITCH + 1

    with tile.TileContext(nc) as tc:
        with (
            tc.tile_pool(name="psum", bufs=1, space="PSUM") as pp,
            tc.tile_pool(name="tmp", bufs=3) as pool,
        ):
            # four PSUM tensors of 2 banks each (tile pairs): fine-grained
            # dependency domains -> short per-pair pipeline loops
            cps = [pp.tile([P, CW], f32, tag=f"c{g}", name=f"c{g}")
                   for g in range(4)]

            # ---------------- init ----------------
            bias_act = nc.alloc_sbuf_tensor("bias_act", [P, 1], f32)
            nc.vector.memset(bias_act.ap(), -s_)
            bias_m = nc.alloc_sbuf_tensor("bias_m", [P, 1], f32)
            nc.vector.memset(bias_m.ap(), -m_)
            for j in range(3):
                nc.sync.dma_start(out=wa16[j].ap(), in_=a16_h[j].ap())
            nc.sync.dma_start(out=wh16.ap(), in_=h16_h.ap())
            nc.sync.dma_start(out=wz16.ap(), in_=z16_h.ap())
            for i in range(2):
                nc.vector.memset(h16b[i].ap(), 0.0)
                nc.vector.memset(dlb[i].ap(), 0.0)

            # load x0, emit state 0
            xv_dram = x_in.rearrange("b (t p) c -> p b t c", p=P)
            nc.sync.dma_start(
                out=xb[0].ap().rearrange("p (b t c) -> p b t c", b=NIMG, t=TPI),
                in_=xv_dram)
            out_v = out.rearrange("s b (t p) c -> p s b t c", p=P)

            def emit_state(x_t, s):
                nc.sync.dma_start(
                    out=out_v[:, s:s + 1],
                    in_=x_t.ap().rearrange(
                        "p (b t c) -> p b t c", b=NIMG, t=TPI).unsqueeze(1))

            emit_state(xb[0], 0)

            CS = 3 * PITCH - 2  # contiguous span of 3 tile windows + pads

            def dl_tile(par, t):
                # [P, W] contiguous view of tile t's delta window
                start = wbase(t)
                return dlb[par].ap()[:, start:start + W]

            def _shift3_src(par, row, base):
                # [1, 3, CS] view of delta row `row`: dim 1 has stride ONE
                # ELEMENT, so dst partition j reads the span shifted by j.
                # One DMA thus writes all 3 pre-shifted halo rows.
                v = dlb[par].ap()[row:row + 1, base:base + CS].unsqueeze(1)
                v = v.copy()
                v.ap[1] = [1, 3]
                return v

            def halo_above(eng, par, b):
                # above-halo rows 0-2 of tiles 1..3 <- row 127 of tiles
                # 0..2, pre-shifted by dc = j-1 via the src j-stride.  The
                # delta pads are permanent zeros and flow into the shifted
                # edges, implementing SAME padding exactly.
                s0 = wbase(b * TPI)
                eng.dma_start(
                    out=h16b[par].ap()[0:3, s0 + PITCH:s0 + PITCH + CS],
                    in_=_shift3_src(par, P - 1, s0 - 1))

            def halo_below(eng, par, b):
                # below-halo rows 3-5 of tiles 0..2 <- row 0 of tiles 1..3
                s0 = wbase(b * TPI)
                eng.dma_start(
                    out=h16b[par].ap()[3:6, s0:s0 + CS],
                    in_=_shift3_src(par, 0, s0 + PITCH - 1))

            def banded_mms(par, pr, start):
                # c[pair pr] += row-banded conv terms of its 2 tiles
                cp = cps[pr]
                for j in (1, 0, 2):
                    dc = j - 1
                    for t in (2 * pr, 2 * pr + 1):
                        ts0, cs0 = wbase(t) + dc, (t % 2) * W
                        nc.tensor.matmul(out=cp[:, cs0:cs0 + W],
                                         lhsT=wa16[j].ap(),
                                         rhs=dlb[par].ap()[:, ts0:ts0 + W],
                                         start=start and j == 1, stop=False)

            def halo_mms(par, pr):
                # boundary-row contributions for pair pr's tiles
                cp = cps[pr]
                for t in (2 * pr, 2 * pr + 1):
                    ts0, cs0 = wbase(t), (t % 2) * W
                    nc.tensor.matmul(out=cp[:, cs0:cs0 + W],
                                     lhsT=wh16.ap(),
                                     rhs=h16b[par].ap()[:, ts0:ts0 + W],
                                     start=False, stop=True)

            def warm_mms(n):
                # zero-weight matmuls into cps[3]: keep the PE clock ramped
                # across the inter-burst gap without touching real state
                for _ in range(n):
                    nc.tensor.matmul(out=cps[3][:, 0:P],
                                     lhsT=wz16.ap(), rhs=wa16[0].ap(),
                                     start=False, stop=False,
                                     skip_group_check=True)

            # fp16 conv of the initial state into PSUM via the delta buffer
            # (parity 1 = "step -1", so step 0's delta writes don't collide)
            for t in range(NT):
                nc.vector.tensor_copy(
                    out=dl_tile(1, t),
                    in_=xb[0].ap()[:, t * W:(t + 1) * W])
            for b in range(NIMG):
                halo_above(nc.sync, 1, b)
                halo_below(nc.sync, 1, b)
            for pr in range(4):
                banded_mms(1, pr, True)
            for pr in range(4):
                halo_mms(1, pr)

            # ---------------- steps ----------------
            def x_update(par, x_cur, x_new, t, eng):
                # x_new = x + delta (f32; 1X on DVE, slower on Pool)
                eng.tensor_tensor(
                    out=x_new.ap()[:, t * W:(t + 1) * W],
                    in0=x_cur.ap()[:, t * W:(t + 1) * W],
                    in1=dl_tile(par, t),
                    op=OP.add)

            for s in range(steps):
                par = s % 2
                x_cur, x_new = xb[s % 4], xb[(s + 1) % 4]
                last = s == steps - 1
                warm_mms(24)

                for pr in range(4):
                    fs = pr * CW
                    u_s = ub.ap()[:, fs:fs + CW]
                    nc.scalar.activation(
                        out=u_s, in_=cps[pr][:, :],
                        func=AF.Derivative_Erf, bias=bias_act.ap(), scale=s_)
                    if pr % 2 == 0:
                        # factored path, all ops in DVE 4x/2x modes:
                        #   dl = [(u-r1)*c2] * [u-r2]
                        w1 = pool.tile([P, CW], f16, tag="w1",
                                       name=f"w1_{s}_{pr}")
                        w2 = pool.tile([P, CW], f16, tag="w2",
                                       name=f"w2_{s}_{pr}")
                        nc.vector.tensor_scalar(
                            out=w1[:], in0=u_s, scalar1=r1_, scalar2=c2_,
                            op0=OP.subtract, op1=OP.mult)
                        nc.vector.tensor_scalar(
                            out=w2[:], in0=u_s, scalar1=r2_, scalar2=1.0,
                            op0=OP.subtract, op1=OP.mult)
                        for ti in range(2):
                            t = 2 * pr + ti
                            nc.vector.tensor_tensor(
                                out=dl_tile(par, t),
                                in0=w1[:, ti * W:(ti + 1) * W],
                                in1=w2[:, ti * W:(ti + 1) * W],
                                op=OP.mult)
                    else:
                        # Square-ACT path (balances ScalarE vs VectorE):
                        #   y = (u-m)^2 on ScalarE, dl = c2*y - c2*d^2
                        y = pool.tile([P, CW], f16, tag="y",
                                      name=f"y_{s}_{pr}")
                        nc.scalar.activation(
                            out=y[:], in_=u_s,
                            func=AF.Square, bias=bias_m.ap(), scale=1.0)
                        for ti in range(2):
                            t = 2 * pr + ti
                            nc.vector.tensor_scalar(
                                out=dl_tile(par, t),
                                in0=y[:, ti * W:(ti + 1) * W],
                                scalar1=c2_, scalar2=cd2_,
                                op0=OP.mult, op1=OP.add)
                    if pr == 1:
                        for t in range(0, 4):
                            x_update(par, x_cur, x_new, t,
                                     nc.vector if t < 2 else nc.gpsimd)
                        if not last:
                            halo_above(nc.sync, par, 0)
                            halo_below(nc.sync, par, 0)
                    elif pr == 3:
                        for t in range(4, 8):
                            x_update(par, x_cur, x_new, t,
                                     nc.vector if t < 6 else nc.gpsimd)
                        if not last:
                            halo_above(nc.sync, par, 1)
                            halo_below(nc.sync, par, 1)
                emit_state(x_new, s + 1)
                if not last:
                    # image-0 pairs close first so the next step's chain
                    # (ACT p0 -> DVE -> banded p0) overlaps this burst's tail
                    for pr in (0, 1):
                        banded_mms(par, pr, False)
                    for pr in (0, 1):
                        halo_mms(par, pr)
                    for pr in (2, 3):
                        banded_mms(par, pr, False)
                    for pr in (2, 3):
                        halo_mms(par, pr)

    nc.compile()
    return nc


# --------------------------------------------------------------------------
# Entry point
# --------------------------------------------------------------------------

def kernel(x, k, w1, b1, w2, steps):
    global LAST_RESULTS
    steps = int(np.asarray(steps))
    x = np.asarray(x, np.float32)
    k = np.asarray(k, np.float32).reshape(3, 3)
    B = x.shape[0]
    assert B == NIMG * NCORES and x.shape[-2:] == (W, W)

    params = _get_params(np.asarray(w1, np.float64), np.asarray(b1, np.float64),
                         np.asarray(w2, np.float64))

    key = (steps, k.tobytes(), tuple(params))
    nc = _NC_CACHE.get(key)
    if nc is None:
        nc = _build_nc(k, params, steps)
        _NC_CACHE.clear()
        _NC_CACHE[key] = nc

    xs = np.ascontiguousarray(x.reshape(B, W, W))
    in_maps = [{"x": np.ascontiguousarray(xs[NIMG * i:NIMG * (i + 1)])}
               for i in range(NCORES)]

    from concourse.bass_utils import run_bass_kernel_spmd
    res = run_bass_kernel_spmd(nc, in_maps, core_ids=list(range(NCORES)))
    LAST_RESULTS = res

    full = np.concatenate([np.asarray(r["out"]) for r in res.results], axis=1)
    return np.ascontiguousarray(full[:, :, None].astype(np.float32))


if __name__ == "__main__":
    rng = np.random.default_rng(0)
    x = rng.standard_normal((16, 1, W, W), dtype=np.float32)
    k = rng.standard_normal((1, 1, 3, 3)).astype(np.float32)
    w1 = (rng.standard_normal((10, 1)) * 0.5).astype(np.float32)
    b1 = (rng.standard_normal((10,)) * 0.1).astype(np.float32)
    w2 = (rng.standard_normal((1, 10)) * 0.5).astype(np.float32)
    out = kernel(x=x, k=k, w1=w1, b1=b1, w2=w2, steps=16)
    print("out", out.shape, out.dtype)


# revision 36
# speedup vs baseline: 1.2515x; 1.2044x over previous
"""Trainium2 Bass kernel for a 16-step neural cellular automaton (BasicNCA).

Reference semantics (per step):
    c   = conv3x3(x, k, SAME)                    # 1 channel
    g   = exp(-(c-1)^2)
    h   = relu(g*w1 + b1); o = sigmoid(h@w2)     # pointwise 1->10->1 MLP
    x  += o - 0.5
Output: all 17 states stacked, [17, 16, 1, 512, 512] f32.

Design (evolution of the previous 291us kernel; see trace analysis):
 * The pointwise chain delta(c) = sigmoid(P(exp(-(c-1)^2))) - 0.5 is an exact
   function of u = exp(-(c-1)^2).  Fitting a quadratic in the Gaussian
        delta(c) ~= c2*(u - r1)*(u - r2),  u = (2/sqrt(pi))*exp(-(s(c-1))^2)
   (refit on the host from the actual w1/b1/w2) has max err 2.2e-3 -- 2.5x
   better than the old Abs+Gelu two-pass form -- and needs only ONE ScalarE
   pass (ActivationFunctionType.Derivative_Erf == (2/sqrt(pi))e^{-x^2}) plus
   two fp16 VectorE ops (tensor_scalar, scalar_tensor_tensor).  The factored
   form makes the VectorE output the FULL delta, so the incremental conv
   needs no constant-drift bookkeeping at all.
 * The conv state c lives permanently in PSUM (all 8 banks) and is updated
   incrementally by the TensorEngine: c += conv3x3(delta) in fp16, as
   3 banded (tridiagonal) 128x128 matmuls per row-tile plus one 6-row halo
   matmul.  delta rows are stored with a 514-element tile pitch whose 2 zero
   pad columns implement SAME-padding column edges for the +-1 shifted
   matmuls, so all matmuls are full 512-column and halo DMAs write full
   unclipped rows.
 * The x update x += delta runs on the otherwise idle Pool/GpSimd engine,
   halo DMAs are split across the sync and pool rings, and the output write
   rides the scalar ring.
 * Sharding: pure data parallel, 2 images per NeuronCore across 8 cores.
"""

import math

import numpy as np

P = 128          # partitions
W = 512          # image width (= free size per row-tile)
TPI = 4          # row-tiles per image (4 * 128 = 512 rows)
NIMG = 2         # images per core
NT = TPI * NIMG  # row-tiles per core
NCORES = 8
FREE = NT * W    # free size of full-state SBUF tensors (x, u)
PITCH = W + 2    # padded tile pitch for delta / halo tensors
FREEP = NT * PITCH + 2  # +2: slack so shifted tile views stay in bounds

# Fitted on the reference setup_inputs() weights; full-trajectory rel err
# 1.5e-3 in a bit-faithful numpy simulation of this kernel.
#   delta(c) ~= c0 + u*(c1 + c2*u), u = (2/sqrt(pi))*exp(-(s*(c-1))^2)
_DEFAULT_PARAMS = (1.08490766, 0.02218426, 0.16743472, -0.01551842)

_NC_CACHE = {}
LAST_RESULTS = None

_K2 = 2.0 / math.sqrt(math.pi)


# --------------------------------------------------------------------------
# Host-side scalar-map fitting
# --------------------------------------------------------------------------

def _delta_exact(c, w1, b1, w2):
    g = np.exp(-(c - 1.0) ** 2)
    z = g[..., None] * w1.reshape(-1) + b1.reshape(-1)
    pv = (np.maximum(z, 0.0) * w2.reshape(-1)).sum(-1)
    return 1.0 / (1.0 + np.exp(-pv)) - 0.5


def _model(p, c):
    s, c0, c1, c2 = p
    u = _K2 * np.exp(-(s * (c - 1.0)) ** 2)
    return c0 + u * (c1 + c2 * u)


def _get_params(w1, b1, w2):
    grid = np.linspace(-26.0, 26.0, 40001)
    target = _delta_exact(grid, w1, b1, w2)
    p0 = np.array(_DEFAULT_PARAMS)
    err0 = float(np.abs(_model(p0, grid) - target).max())
    if err0 < 4e-3:
        return tuple(p0)
    # Weights differ from the ones this kernel was tuned on -- refit.
    tail = float(target[0])
    best = (err0, p0)
    try:
        from scipy.optimize import least_squares
        for s0 in (0.6, 1.0, 1.6):
            peak = float(target[grid.searchsorted(1.0)])
            c1g = (peak - tail) / _K2
            init = [s0, tail, c1g, 0.0]
            try:
                sol = least_squares(lambda p: _model(p, grid) - target,
                                    init, max_nfev=8000)
                e = float(np.abs(_model(sol.x, grid) - target).max())
                if e < best[0]:
                    best = (e, sol.x)
            except Exception:
                pass
    except Exception:
        pass
    return tuple(float(v) for v in best[1])


# --------------------------------------------------------------------------
# Bass program
# --------------------------------------------------------------------------

def _build_nc(kk, params, steps):
    from concourse import bacc, mybir, tile

    f32 = mybir.dt.float32
    f16 = mybir.dt.float16
    AF = mybir.ActivationFunctionType
    OP = mybir.AluOpType

    s_, c0_, c1_, c2_ = [float(v) for v in params]
    # delta = c0 + c1*u + c2*u^2 = c2*(u - r1)*(u - r2); complex roots can
    # only arise from a degenerate refit -- nudge c0 until real.
    disc = c1_ * c1_ - 4.0 * c2_ * c0_
    if disc < 0.0:
        c0_ = c1_ * c1_ / (4.0 * c2_) * 0.999
        disc = c1_ * c1_ - 4.0 * c2_ * c0_
    r1_ = (-c1_ + math.sqrt(disc)) / (2.0 * c2_)
    r2_ = (-c1_ - math.sqrt(disc)) / (2.0 * c2_)
    # complete-the-square form for the Square-ACT path:
    #   delta = c2*(u - m)^2 - c2*d^2
    m_ = (r1_ + r2_) / 2.0
    cd2_ = -c2_ * ((r1_ - r2_) / 2.0) ** 2

    kk = np.asarray(kk, np.float32).reshape(3, 3)
    kk16 = kk.astype(np.float16)

    nc = bacc.Bacc("TRN2", target_bir_lowering=False, debug=False,
                   num_devices=NCORES)
    x_in = nc.dram_tensor("x", [NIMG, W, W], f32, kind="ExternalInput")
    out = nc.dram_tensor("out", [steps + 1, NIMG, W, W], f32,
                         kind="ExternalOutput")

    # ---- host-built constants --------------------------------------------
    def banded(kcol):
        # lhsT[qrow, prow]: input row q feeds output row p with kernel row
        # index 1 + (q - p).  out[p,c] = sum_q lhsT[q,p] * rhs[q,c].
        m = np.zeros((P, P), kcol.dtype)
        for dr in (-1, 0, 1):
            for p in range(P):
                q = p + dr
                if 0 <= q < P:
                    m[q, p] = kcol[1 + dr]
        return m

    a16_h = [nc.inline_tensor(banded(kk16[:, j]), name=f"A16{j}")
             for j in range(3)]
    z16_h = nc.inline_tensor(np.zeros((P, P), np.float16), name="Z16")

    # Shared 6-row halo lhsT: rows 0-2 above-halo (k[0,j] -> out row 0),
    # rows 3-5 below-halo (k[2,j] -> out row 127).
    hm = np.zeros((6, P), np.float16)
    for j in range(3):
        hm[j, 0] = kk16[0, j]
        hm[3 + j, P - 1] = kk16[2, j]
    h16_h = nc.inline_tensor(hm, name="H16")

    # ---- on-chip tensors -------------------------------------------------
    # 4-deep x rotation: the emit DMA of state s has 3 full steps to drain
    # before its buffer is rewritten, so the x update never blocks on it.
    xb = [nc.alloc_sbuf_tensor(f"xs{i}", [P, FREE], f32) for i in range(4)]
    ub = nc.alloc_sbuf_tensor("u16", [P, FREE], f16)
    # dl / h16 double-buffered by step parity so this step's delta writes
    # never wait on the previous conv burst's reads
    dlb = [nc.alloc_sbuf_tensor(f"delta{i}", [P, FREEP], f16)
           for i in range(2)]
    h16b = [nc.alloc_sbuf_tensor(f"halo16_{i}", [6, FREEP], f16)
            for i in range(2)]
    wa16 = [nc.alloc_sbuf_tensor(f"wa16{j}", [P, P], f16) for j in range(3)]
    wz16 = nc.alloc_sbuf_tensor("wz16", [P, P], f16)
    wh16 = nc.alloc_sbuf_tensor("wh16", [6, P], f16)

    CW = 2 * W  # pointwise chunk = one PSUM pair (2 tiles)

    def wbase(bt):
        return bt * PITCH + 1

    with tile.TileContext(nc) as tc:
        with (
            tc.tile_pool(name="psum", bufs=1, space="PSUM") as pp,
            tc.tile_pool(name="tmp", bufs=3) as pool,
        ):
            # four PSUM tensors of 2 banks each (tile pairs): fine-grained
            # dependency domains -> short per-pair pipeline loops
            cps = [pp.tile([P, CW], f32, tag=f"c{g}", name=f"c{g}")
                   for g in range(4)]

            # ---------------- init ----------------
            bias_act = nc.alloc_sbuf_tensor("bias_act", [P, 1], f32)
            nc.vector.memset(bias_act.ap(), -s_)
            bias_m = nc.alloc_sbuf_tensor("bias_m", [P, 1], f32)
            nc.vector.memset(bias_m.ap(), -m_)
            for j in range(3):
                nc.sync.dma_start(out=wa16[j].ap(), in_=a16_h[j].ap())
            nc.sync.dma_start(out=wh16.ap(), in_=h16_h.ap())
            nc.sync.dma_start(out=wz16.ap(), in_=z16_h.ap())
            for i in range(2):
                nc.vector.memset(h16b[i].ap(), 0.0)
                nc.vector.memset(dlb[i].ap(), 0.0)

            # load x0 as 4 per-pair descriptors (parallel DMA queues),
            # emit state 0
            xv_dram = x_in.rearrange("b (t p) c -> p b t c", p=P)
            xb0v = xb[0].ap().rearrange("p (b t c) -> p b t c", b=NIMG, t=TPI)
            for pr in range(4):
                b, tp = pr // 2, 2 * (pr % 2)
                nc.sync.dma_start(out=xb0v[:, b:b + 1, tp:tp + 2],
                                  in_=xv_dram[:, b:b + 1, tp:tp + 2])
            out_v = out.rearrange("s b (t p) c -> p s b t c", p=P)

            def emit_state(x_t, s, split=False):
                xv = x_t.ap().rearrange(
                    "p (b t c) -> p b t c", b=NIMG, t=TPI).unsqueeze(1)
                if split:
                    for b in range(NIMG):
                        nc.sync.dma_start(out=out_v[:, s:s + 1, b:b + 1],
                                          in_=xv[:, :, b:b + 1])
                else:
                    nc.sync.dma_start(out=out_v[:, s:s + 1], in_=xv)

            emit_state(xb[0], 0)

            CS = 3 * PITCH - 2  # contiguous span of 3 tile windows + pads

            def dl_tile(par, t):
                # [P, W] contiguous view of tile t's delta window
                start = wbase(t)
                return dlb[par].ap()[:, start:start + W]

            def _shift3_src(par, row, base):
                # [1, 3, CS] view of delta row `row`: dim 1 has stride ONE
                # ELEMENT, so dst partition j reads the span shifted by j.
                # One DMA thus writes all 3 pre-shifted halo rows.
                v = dlb[par].ap()[row:row + 1, base:base + CS].unsqueeze(1)
                v = v.copy()
                v.ap[1] = [1, 3]
                return v

            def halo_above(eng, par, b):
                # above-halo rows 0-2 of tiles 1..3 <- row 127 of tiles
                # 0..2, pre-shifted by dc = j-1 via the src j-stride.  The
                # delta pads are permanent zeros and flow into the shifted
                # edges, implementing SAME padding exactly.
                s0 = wbase(b * TPI)
                eng.dma_start(
                    out=h16b[par].ap()[0:3, s0 + PITCH:s0 + PITCH + CS],
                    in_=_shift3_src(par, P - 1, s0 - 1))

            def halo_below(eng, par, b):
                # below-halo rows 3-5 of tiles 0..2 <- row 0 of tiles 1..3
                s0 = wbase(b * TPI)
                eng.dma_start(
                    out=h16b[par].ap()[3:6, s0:s0 + CS],
                    in_=_shift3_src(par, 0, s0 + PITCH - 1))

            def banded_mms(par, pr, start):
                # c[pair pr] += row-banded conv terms of its 2 tiles
                cp = cps[pr]
                for j in (1, 0, 2):
                    dc = j - 1
                    for t in (2 * pr, 2 * pr + 1):
                        ts0, cs0 = wbase(t) + dc, (t % 2) * W
                        nc.tensor.matmul(out=cp[:, cs0:cs0 + W],
                                         lhsT=wa16[j].ap(),
                                         rhs=dlb[par].ap()[:, ts0:ts0 + W],
                                         start=start and j == 1, stop=False)

            def halo_mms(par, pr):
                # boundary-row contributions for pair pr's tiles
                cp = cps[pr]
                for t in (2 * pr, 2 * pr + 1):
                    ts0, cs0 = wbase(t), (t % 2) * W
                    nc.tensor.matmul(out=cp[:, cs0:cs0 + W],
                                     lhsT=wh16.ap(),
                                     rhs=h16b[par].ap()[:, ts0:ts0 + W],
                                     start=False, stop=True)

            def warm_mms(n):
                # zero-weight matmuls into cps[3]: keep the PE clock ramped
                # across the inter-burst gap without touching real state
                for _ in range(n):
                    nc.tensor.matmul(out=cps[3][:, 0:P],
                                     lhsT=wz16.ap(), rhs=wa16[0].ap(),
                                     start=False, stop=False,
                                     skip_group_check=True)

            warm_mms(80)
            # fp16 conv of the initial state into PSUM via the delta buffer
            # (parity 1 = "step -1", so step 0's delta writes don't collide)
            for t in range(NT):
                nc.vector.tensor_copy(
                    out=dl_tile(1, t),
                    in_=xb[0].ap()[:, t * W:(t + 1) * W])
            for b in range(NIMG):
                halo_above(nc.sync, 1, b)
                halo_below(nc.sync, 1, b)
            for pr in range(4):
                banded_mms(1, pr, True)
            for pr in range(4):
                halo_mms(1, pr)

            # ---------------- steps ----------------
            def x_update(par, x_cur, x_new, t, eng):
                # x_new = x + delta (f32; 1X on DVE, slower on Pool)
                eng.tensor_tensor(
                    out=x_new.ap()[:, t * W:(t + 1) * W],
                    in0=x_cur.ap()[:, t * W:(t + 1) * W],
                    in1=dl_tile(par, t),
                    op=OP.add)

            for s in range(steps):
                par = s % 2
                x_cur, x_new = xb[s % 4], xb[(s + 1) % 4]
                last = s == steps - 1
                warm_mms(24)

                for pr in range(4):
                    fs = pr * CW
                    u_s = ub.ap()[:, fs:fs + CW]
                    nc.scalar.activation(
                        out=u_s, in_=cps[pr][:, :],
                        func=AF.Derivative_Erf, bias=bias_act.ap(), scale=s_)
                    if pr % 2 == 0:
                        # factored path, all ops in DVE 4x/2x modes:
                        #   dl = [(u-r1)*c2] * [u-r2]
                        w1 = pool.tile([P, CW], f16, tag="w1",
                                       name=f"w1_{s}_{pr}")
                        w2 = pool.tile([P, CW], f16, tag="w2",
                                       name=f"w2_{s}_{pr}")
                        nc.vector.tensor_scalar(
                            out=w1[:], in0=u_s, scalar1=r1_, scalar2=c2_,
                            op0=OP.subtract, op1=OP.mult)
                        nc.vector.tensor_scalar(
                            out=w2[:], in0=u_s, scalar1=r2_, scalar2=1.0,
                            op0=OP.subtract, op1=OP.mult)
                        for ti in range(2):
                            t = 2 * pr + ti
                            nc.vector.tensor_tensor(
                                out=dl_tile(par, t),
                                in0=w1[:, ti * W:(ti + 1) * W],
                                in1=w2[:, ti * W:(ti + 1) * W],
                                op=OP.mult)
                    else:
                        # Square-ACT path (balances ScalarE vs VectorE):
                        #   y = (u-m)^2 on ScalarE, dl = c2*y - c2*d^2
                        y = pool.tile([P, CW], f16, tag="y",
                                      name=f"y_{s}_{pr}")
                        nc.scalar.activation(
                            out=y[:], in_=u_s,
                            func=AF.Square, bias=bias_m.ap(), scale=1.0)
                        for ti in range(2):
                            t = 2 * pr + ti
                            nc.vector.tensor_scalar(
                                out=dl_tile(par, t),
                                in0=y[:, ti * W:(ti + 1) * W],
                                scalar1=c2_, scalar2=cd2_,
                                op0=OP.mult, op1=OP.add)
                    if pr == 1:
                        for t in range(0, 4):
                            x_update(par, x_cur, x_new, t,
                                     nc.vector if t < 2 else nc.gpsimd)
                        if not last:
                            halo_above(nc.sync, par, 0)
                            halo_below(nc.sync, par, 0)
                    elif pr == 3:
                        for t in range(4, 8):
                            x_update(par, x_cur, x_new, t,
                                     nc.vector if t < 6 else nc.gpsimd)
                        if not last:
                            halo_above(nc.sync, par, 1)
                            halo_below(nc.sync, par, 1)
                emit_state(x_new, s + 1, split=last)
                if not last:
                    # image-0 pairs close first so the next step's chain
                    # (ACT p0 -> DVE -> banded p0) overlaps this burst's tail
                    for pr in (0, 1):
                        banded_mms(par, pr, False)
                    for pr in (0, 1):
                        halo_mms(par, pr)
                    for pr in (2, 3):
                        banded_mms(par, pr, False)
                    for pr in (2, 3):
                        halo_mms(par, pr)

    nc.compile()
    return nc


# --------------------------------------------------------------------------
# Entry point
# --------------------------------------------------------------------------

def kernel(x, k, w1, b1, w2, steps):
    global LAST_RESULTS
    steps = int(np.asarray(steps))
    x = np.asarray(x, np.float32)
    k = np.asarray(k, np.float32).reshape(3, 3)
    B = x.shape[0]
    assert B == NIMG * NCORES and x.shape[-2:] == (W, W)

    params = _get_params(np.asarray(w1, np.float64), np.asarray(b1, np.float64),
                         np.asarray(w2, np.float64))

    key = (steps, k.tobytes(), tuple(params))
    nc = _NC_CACHE.get(key)
    if nc is None:
        nc = _build_nc(k, params, steps)
        _NC_CACHE.clear()
        _NC_CACHE[key] = nc

    xs = np.ascontiguousarray(x.reshape(B, W, W))
    in_maps = [{"x": np.ascontiguousarray(xs[NIMG * i:NIMG * (i + 1)])}
               for i in range(NCORES)]

    from concourse.bass_utils import run_bass_kernel_spmd
    res = run_bass_kernel_spmd(nc, in_maps, core_ids=list(range(NCORES)))
    LAST_RESULTS = res

    full = np.concatenate([np.asarray(r["out"]) for r in res.results], axis=1)
    return np.ascontiguousarray(full[:, :, None].astype(np.float32))


if __name__ == "__main__":
    rng = np.random.default_rng(0)
    x = rng.standard_normal((16, 1, W, W), dtype=np.float32)
    k = rng.standard_normal((1, 1, 3, 3)).astype(np.float32)
    w1 = (rng.standard_normal((10, 1)) * 0.5).astype(np.float32)
    b1 = (rng.standard_normal((10,)) * 0.1).astype(np.float32)
    w2 = (rng.standard_normal((1, 10)) * 0.5).astype(np.float32)
    out = kernel(x=x, k=k, w1=w1, b1=b1, w2=w2, steps=16)
    print("out", out.shape, out.dtype)
